# revision 1
# baseline (speedup 1.0000x reference)
"""Trainium2 Bass kernel for nn_BuddyPool_42537356100368 (retrieval_knn) — v2.

kernel(cue, patches) -> (16, 5, 1024) f32: for each (example, cue) pair, mean
of the 9 L2-normalized patches most cosine-similar to the cue.

Sharding: pure data parallel — batch dim 16 split as 2 examples per core
across 8 NeuronCores. Self-contained: shapes hardcoded.

v2 changes vs baseline (205 us -> 146 us measured, iterating):
  - fp32->fp16 cast folded into the patch-stream DMA (SWDGE cast on gpsimd),
    freeing DVE/ACT of ~50 us of cast work.
  - top-k restructured: per-512-chunk top-8 (max8+max_index) runs *inside*
    the stream loop, hidden under the patch DMA; the serial full-width
    9-pass top-24 tail (~39 us) is gone.  Final merge packs (clamped
    quantized value, index) into one fp32 word <= 2^24-1 (exact in fp32),
    takes top-24 with 3 max8 rounds, decodes the index with an integer AND.
  - exact rescore: norms via ACT Square+accum, dots via DVE fused
    scalar_tensor_tensor with accum_out.
  - candidate-weight assembly via a constant mask multiply instead of
    small serial DMAs.
  - per-example tail emitted right after that example's stream so e0's
    merge/gather/rescore overlaps e1's streaming (interleave=True).

Phase B exact fp32 rescore of 24 candidates/cue: output matches the fp32
reference to ~3e-7 relative (verified on HW).
"""
from contextlib import ExitStack

import numpy as np

import concourse.bass as bass
import concourse.bacc as bacc
import concourse.mybir as mybir
import concourse.tile as tile
from concourse.bass_utils import run_bass_kernel_spmd
from concourse.masks import make_identity

FP32 = mybir.dt.float32
FP16 = mybir.dt.float16
I32 = mybir.dt.int32
U32 = mybir.dt.uint32
NEG_BIG = -3.0e38

B, K, N, D = 16, 5, 4096, 1024
NCORES = 8
EB = B // NCORES  # examples per core
C = 24            # exact-rescore candidates per cue row

LAST_EXEC_NS = None
_CACHE = {}


def _split_multiwaits(nc):
    """The walrus build in this container rejects >1 sem-wait per instruction
    (setupSyncWait assert); hoist extra waits onto preceding NoOps."""
    cnt = 0
    for f in nc.m.functions:
        for bb in f.blocks:
            insts = list(bb.instructions)
            if not any(
                i.sync_info and i.sync_info.on_wait and len(i.sync_info.on_wait) > 1
                for i in insts
            ):
                continue
            new_list = []
            for ins in insts:
                si = ins.sync_info
                if si and si.on_wait and len(si.on_wait) > 1:
                    waits = list(si.on_wait)
                    for w in waits[:-1]:
                        cnt += 1
                        nop = mybir.InstNoOp(
                            name=f"W-split-{cnt}", engine=ins.engine, ins=[], outs=[]
                        )
                        nop.sync_info = mybir.SyncInfo(on_wait=[w], on_update=[])
                        new_list.append(nop)
                    ins.sync_info = mybir.SyncInfo(
                        on_wait=[waits[-1]], on_update=list(si.on_update)
                    )
                new_list.append(ins)
            bb.instructions = new_list
    return cnt


def _build_kernel(split=True, loop_iters=None, cast_dma=True, dbg=False,
                  tr_mode="tr", interleave=False, tail=True, nsup=1024,
                  act_copies=6, f16_bufs=None, ps_split=(4, 2),
                  coalesce=False):
    # coalesce: map 8 *consecutive* patch rows to each partition so each DMA
    # descriptor covers 32 KB of contiguous DRAM (8x fewer descriptors).
    # Induces a col->patch permutation n = (col&127)*8 + (col>>7) + base,
    # undone with integer ops at merge time. Forces uniform super-tiles.
    # act_copies: how many of the 8 per-super-tile pt PSUM->SBUF copies go
    # to the scalar (ACT) engine; the rest go to DVE.
    # tr_mode: "tr" = PE transpose-mode instructions (fp16 PSUM out);
    #          "mm" = regular fp16 matmul vs identity (fp32 PSUM out) —
    #          LDWEIGHTS pipelines through the PE reorder window and the
    #          matmuls keep HAM at 2.4 GHz, at the price of fp32 PSUM
    #          (2 banks per transpose tile) and 1x-mode PSUM copies.
    # interleave: emit example e's merge/rescore tail right after its
    #          stream so it overlaps the next example's streaming.
    # tail=False: stream+chunk-topk only (bench diagnostics).
    nc = bacc.Bacc("TRN2", target_bir_lowering=False, debug=False)
    cue_d = nc.dram_tensor("cue", [EB, K, D], FP32, kind="ExternalInput")
    pat_d = nc.dram_tensor("patches", [EB, N, D], FP32,
                           kind="Internal" if loop_iters else "ExternalInput")
    out_d = nc.dram_tensor("out", [EB, K, D], FP32, kind="ExternalOutput")
    if dbg:
        dbg_v = nc.dram_tensor("dbg_v", [EB, K, 64], FP32, kind="ExternalOutput")
        dbg_ix = nc.dram_tensor("dbg_ix", [EB, K, 64], U32, kind="ExternalOutput")
        dbg_n24 = nc.dram_tensor("dbg_n24", [EB, K, C], U32, kind="ExternalOutput")
        dbg_dots = nc.dram_tensor("dbg_dots", [EB, K * C], FP32, kind="ExternalOutput")
        dbg_nrm = nc.dram_tensor("dbg_nrm", [EB, K * C], FP32, kind="ExternalOutput")
        dbg_mask = nc.dram_tensor("dbg_mask", [EB, K, C], FP32, kind="ExternalOutput")
        dbg_enc = nc.dram_tensor("dbg_enc", [EB, K, 64], FP32, kind="ExternalOutput")
        dbg_e24 = nc.dram_tensor("dbg_e24", [EB, K, C], FP32, kind="ExternalOutput")

    DJ = D // 128          # d-chunks of 128
    NSUP = nsup            # n span per super-tile (4 MB fp32 read at 1024)
    NCH = NSUP // 128      # 128-row groups per super-tile
    NSUPS = N // NSUP      # super-tiles per example
    CHUNK = 512            # top-k chunk width
    NCHUNKS = N // CHUNK   # candidate chunks per example (8)
    M = NCHUNKS * 8        # merged candidates per cue row (64)
    KC = K * C             # gathered candidate rows per example (120 <= 128)

    with tile.TileContext(nc) as tc, ExitStack() as ctx:
        if f16_bufs is None:
            f16_bufs = (4 if nsup <= 1024 else 2) if cast_dma else 2
        p_f16 = ctx.enter_context(tc.tile_pool(name="f16", bufs=f16_bufs))
        if not cast_dma:
            p_raw = ctx.enter_context(tc.tile_pool(name="raw", bufs=2))
        p_pt = ctx.enter_context(tc.tile_pool(name="pt", bufs=2))
        p_sim = ctx.enter_context(tc.tile_pool(name="sim", bufs=3))
        p_persist = ctx.enter_context(tc.tile_pool(name="persist", bufs=1))
        p_small = ctx.enter_context(tc.tile_pool(name="small", bufs=2))
        tr_bufs = ps_split[0] if (tr_mode == "tr" and nsup <= 1024) else 2
        p_ps_tr = ctx.enter_context(tc.tile_pool(
            name="ps_tr", bufs=tr_bufs, space="PSUM"))
        p_ps_s = ctx.enter_context(tc.tile_pool(
            name="ps_s", bufs=ps_split[1], space="PSUM"))
        p_ps_m = ctx.enter_context(tc.tile_pool(name="ps_m", bufs=2, space="PSUM"))

        id128_16 = p_persist.tile([128, 128], FP16, tag="id128_16")
        make_identity(nc, id128_16[:])
        idK_16 = p_persist.tile([K, K], FP16, tag="idK_16")
        make_identity(nc, idK_16[:])

        cueT = [p_persist.tile([128, DJ, K], FP16, tag=f"cueT_{e}", name=f"cueT_{e}")
                for e in range(EB)]
        cue_sb = [p_persist.tile([K, D], FP32, tag=f"cue_sb_{e}", name=f"cue_sb_{e}")
                  for e in range(EB)]
        cue_bc = [p_persist.tile([KC, D], FP32, tag=f"cbc_{e}", name=f"cbc_{e}")
                  for e in range(EB)]

        # constant masks: sel (cue replication), bdm (weight scatter)
        sel = p_persist.tile([K, KC], FP32, tag="sel")
        bdm = p_persist.tile([KC, K], FP32, tag="bdm")
        ones_r = p_persist.tile([1, C], FP32, tag="ones_r")
        ones_c = p_persist.tile([C, 1], FP32, tag="ones_c")
        nc.vector.memset(sel[:], 0.0)
        nc.vector.memset(bdm[:], 0.0)
        nc.vector.memset(ones_r[:], 1.0)
        nc.vector.memset(ones_c[:], 1.0)
        for k in range(K):
            nc.scalar.dma_start(out=sel[k:k + 1, C * k:C * (k + 1)], in_=ones_r[:])
            nc.scalar.dma_start(out=bdm[C * k:C * (k + 1), k:k + 1], in_=ones_c[:])

        # cue prep: load, cast fp16, PE-transpose into [d, k] chunks;
        # cue_bc = sel.T @ cue (replicate each cue row 24x) for the rescore.
        for e in range(EB):
            nc.scalar.dma_start(out=cue_sb[e][:], in_=cue_d.ap()[e])
            c16 = p_small.tile([K, D], FP16, tag="c16")
            nc.vector.tensor_copy(c16[:], cue_sb[e][:])
            for j in range(DJ):
                pst = p_ps_m.tile([128, K], FP16, space="PSUM", tag="ps_misc")
                nc.tensor.matmul(
                    pst[:], c16[:, 128 * j:128 * (j + 1)], idK_16[:],
                    is_transpose=True, start=True, stop=True,
                )
                nc.vector.tensor_copy(cueT[e][:, j, :], pst[:])
            for h in range(D // 512):
                ps_c = p_ps_m.tile([KC, 512], FP32, space="PSUM", tag="ps_misc")
                nc.tensor.matmul(
                    ps_c[:], sel[:], cue_sb[e][:, 512 * h:512 * (h + 1)],
                    start=True, stop=True,
                )
                nc.scalar.copy(cue_bc[e][:, 512 * h:512 * (h + 1)], ps_c[:])

        # per-row candidate accumulators
        v_e = [p_persist.tile([K, M], FP32, tag=f"v_{e}", name=f"v_{e}")
               for e in range(EB)]
        idx_e = [p_persist.tile([K, M], U32, tag=f"ix_{e}", name=f"ix_{e}")
                 for e in range(EB)]

        pat_flat = pat_d.ap().rearrange("e n d -> (e n) d")
        gath = [p_persist.tile([KC, D], FP32, tag=f"g_{e}", name=f"g_{e}")
                for e in range(EB)]
        idx_col = [p_persist.tile([KC, 1], U32, tag=f"ic_{e}", name=f"ic_{e}")
                   for e in range(EB)]
        nrm = [p_persist.tile([KC, 1], FP32, tag=f"nr_{e}", name=f"nr_{e}")
               for e in range(EB)]
        dots = [p_persist.tile([KC, 1], FP32, tag=f"do_{e}", name=f"do_{e}")
                for e in range(EB)]
        outsb = [p_persist.tile([K, D], FP32, tag=f"ou_{e}", name=f"ou_{e}")
                 for e in range(EB)]
        junk = [p_persist.tile([KC, D], FP32, tag=f"jk{e}", name=f"jk{e}")
                for e in range(EB)]
        junk2 = [p_persist.tile([KC, D], FP32, tag=f"jl{e}", name=f"jl{e}")
                 for e in range(EB)]

        # coalesce mode: per-chunk base constants (n0_supertile + 4*(chunk%2)
        # + e*N), one [K, 8]-wide memset per chunk slot group.
        if coalesce:
            baseco = [p_persist.tile([K, M], FP32, tag=f"bc_{e}", name=f"bco_{e}")
                      for e in range(EB)]
            for e in range(EB):
                for cg in range(NCHUNKS):
                    nc.gpsimd.memset(
                        baseco[e][:, 8 * cg:8 * cg + 8],
                        float(1024 * (cg // 2) + 4 * (cg % 2) + e * N))

        def _stream(e, sizes):
            # stream patches; transpose; dot with cues; per-chunk top-8
            n0 = 0
            for sz in sizes:
                nch = sz // 128
                if coalesce:
                    src = pat_d.ap()[e, n0:n0 + sz].rearrange(
                        "(p c) d -> p c d", c=nch)
                else:
                    src = pat_d.ap()[e, n0:n0 + sz].rearrange(
                        "(c p) d -> p c d", p=128)
                f16 = p_f16.tile([128, nch, D], FP16, tag="f16",
                                 padded_shape=[128, NCH, D])
                if cast_dma:
                    with tc.high_priority():
                        nc.gpsimd.dma_start(out=f16[:], in_=src)
                else:
                    raw = p_raw.tile([128, nch, D], FP32, tag="raw",
                                     padded_shape=[128, NCH, D])
                    # alternate the two HWDGE rings for deeper DMA pipelining
                    heng = nc.sync if (n0 // NSUP + e) % 2 == 0 else nc.scalar
                    with tc.high_priority():
                        heng.dma_start(out=raw[:], in_=src)
                    # split the cast: half DVE, half ACT
                    nc.vector.tensor_copy(f16[:, 0:nch // 2, :],
                                          raw[:, 0:nch // 2, :])
                    nc.scalar.copy(f16[:, nch // 2:, :], raw[:, nch // 2:, :])
                pt = p_pt.tile([128, DJ, sz], FP16, tag="pt",
                               padded_shape=[128, DJ, NSUP])
                for j in range(DJ):
                    pst = p_ps_tr.tile([128, sz],
                                       FP16 if tr_mode == "tr" else FP32,
                                       space="PSUM", tag="ps_tr",
                                       padded_shape=[128, NSUP])
                    for c in range(nch):
                        nc.tensor.matmul(
                            pst[:, 128 * c:128 * (c + 1)],
                            f16[:, c, 128 * j:128 * (j + 1)],
                            id128_16[:],
                            is_transpose=(tr_mode == "tr"),
                            start=True, stop=True,
                        )
                    if j >= act_copies:
                        nc.vector.tensor_copy(pt[:, j, :], pst[:])
                    else:
                        nc.scalar.copy(pt[:, j, :], pst[:])
                for h in range(sz // CHUNK):
                    ps_s = p_ps_s.tile([K, CHUNK], FP32, space="PSUM",
                                       tag="ps_s")
                    for j in range(DJ):
                        nc.tensor.matmul(
                            ps_s[:], cueT[e][:, j, :],
                            pt[:, j, CHUNK * h:CHUNK * (h + 1)],
                            start=(j == 0), stop=(j == DJ - 1),
                        )
                    simch = p_sim.tile([K, CHUNK], FP32, tag="simch")
                    nc.scalar.copy(simch[:], ps_s[:])
                    cg = (n0 + h * CHUNK) // CHUNK  # global chunk id
                    nc.vector.max(v_e[e][:, 8 * cg:8 * cg + 8], simch[:])
                    nc.vector.max_index(
                        idx_e[e][:, 8 * cg:8 * cg + 8],
                        v_e[e][:, 8 * cg:8 * cg + 8], simch[:])
                    if not coalesce:
                        nc.vector.tensor_scalar_add(
                            idx_e[e][:, 8 * cg:8 * cg + 8],
                            idx_e[e][:, 8 * cg:8 * cg + 8],
                            e * N + cg * CHUNK)
                n0 += sz

        def _tail_merge(e):
            # merge: pack (clamped quantized value, index) into one fp32
            # word, top-24 via 3 max8 rounds, decode index via integer AND.
            # DVE + one small HWDGE DMA only — safe to emit before the next
            # example's stream (keeps the Pool/SWDGE queue unblocked).
            ev = nc.vector
            q_i = p_small.tile([K, M], I32, tag="q_i", name=f"q_i{e}")
            q_t = p_small.tile([K, M], FP32, tag="q_t", name=f"q_t{e}")
            q_f = p_small.tile([K, M], FP32, tag="q_f", name=f"q_f{e}")
            enc = p_small.tile([K, M], FP32, tag="enc", name=f"enc{e}")
            scr = p_small.tile([K, M], FP32, tag="scr", name=f"scr{e}")
            idxf = p_small.tile([K, M], FP32, tag="idxf", name=f"idxf{e}")
            e24 = p_small.tile([K, C], FP32, tag="e24", name=f"e24{e}")
            e24i = p_small.tile([K, C], U32, tag="e24i", name=f"e24i{e}")
            n24 = p_small.tile([K, C], U32, tag="n24", name=f"n24{e}")
            # q = clamp(trunc(v*4), -1023, 1023); raw dots reach ~±160 so
            # quantization step 0.25 keeps (q+1023)*8192+idx <= 2^24-1,
            # exact in fp32 (index bits survive the max8 rounds).
            ev.tensor_scalar(q_t[:], v_e[e][:], 4.0, 1023.0,
                             op0=mybir.AluOpType.mult,
                             op1=mybir.AluOpType.min)
            ev.tensor_scalar_max(q_t[:], q_t[:], -1023.0)
            ev.tensor_copy(q_i[:], q_t[:])
            ev.tensor_copy(q_f[:], q_i[:])
            ev.tensor_scalar(enc[:], q_f[:], 8192.0, 1023.0 * 8192.0,
                             op0=mybir.AluOpType.mult,
                             op1=mybir.AluOpType.add)
            if coalesce:
                # undo the (p c) load permutation: n = (col&127)*8 + (col>>7)
                # + base(chunk), then enc += n
                c_and = p_small.tile([K, M], U32, tag="c_and", name=f"ca{e}")
                c_shr = p_small.tile([K, M], U32, tag="c_shr", name=f"cs{e}")
                ev.tensor_scalar(c_and[:], idx_e[e][:], 127, 3,
                                 op0=mybir.AluOpType.bitwise_and,
                                 op1=mybir.AluOpType.logical_shift_left)
                ev.tensor_scalar(c_shr[:], idx_e[e][:], 7, None,
                                 op0=mybir.AluOpType.logical_shift_right)
                ev.tensor_tensor(c_and[:], c_and[:], c_shr[:],
                                 op=mybir.AluOpType.add)
                ev.tensor_copy(idxf[:], c_and[:])
                ev.tensor_tensor(idxf[:], idxf[:], baseco[e][:],
                                 op=mybir.AluOpType.add)
            else:
                ev.tensor_copy(idxf[:], idx_e[e][:])
            ev.tensor_tensor(enc[:], enc[:], idxf[:],
                             op=mybir.AluOpType.add)
            if dbg:
                nc.sync.dma_start(out=dbg_enc.ap()[e], in_=enc[:])
            nc.vector.max(e24[:, 0:8], enc[:])
            nc.vector.match_replace(scr[:], e24[:, 0:8], enc[:], NEG_BIG)
            nc.vector.max(e24[:, 8:16], scr[:])
            nc.vector.match_replace(enc[:], e24[:, 8:16], scr[:], NEG_BIG)
            nc.vector.max(e24[:, 16:24], enc[:])
            if dbg:
                nc.sync.dma_start(out=dbg_e24.ap()[e], in_=e24[:])
            ev.tensor_copy(e24i[:], e24[:])
            ev.tensor_scalar(n24[:], e24i[:], 8191, None,
                             op0=mybir.AluOpType.bitwise_and)
            if dbg:
                nc.sync.dma_start(out=dbg_v.ap()[e], in_=v_e[e][:])
                nc.sync.dma_start(out=dbg_ix.ap()[e], in_=idx_e[e][:])
                nc.sync.dma_start(out=dbg_n24.ap()[e], in_=n24[:])
            nc.scalar.dma_start(out=idx_col[e][:], in_=n24[:])

        def _tail_rest(e):
            nc.gpsimd.indirect_dma_start(
                out=gath[e][:],
                out_offset=None,
                in_=pat_flat,
                in_offset=bass.IndirectOffsetOnAxis(
                    ap=idx_col[e][:, :1], axis=0),
            )

            # exact fp32 rescore: cosine = dot * rsqrt(norm2)
            nc.scalar.activation(
                junk[e][:], gath[e][:],
                mybir.ActivationFunctionType.Square,
                accum_out=nrm[e][:])
            nc.vector.scalar_tensor_tensor(
                junk2[e][:], gath[e][:], 1.0, cue_bc[e][:],
                op0=mybir.AluOpType.mult, op1=mybir.AluOpType.mult,
                accum_out=dots[e][:])
            nc.scalar.sqrt(nrm[e][:], nrm[e][:])
            nc.vector.reciprocal(nrm[e][:], nrm[e][:])
            nc.vector.tensor_tensor(dots[e][:], dots[e][:], nrm[e][:],
                                    op=mybir.AluOpType.mult)
            if dbg:
                nc.sync.dma_start(
                    out=dbg_dots.ap()[e].rearrange("(kc one) -> kc one", one=1),
                    in_=dots[e][:])
                nc.sync.dma_start(
                    out=dbg_nrm.ap()[e].rearrange("(kc one) -> kc one", one=1),
                    in_=nrm[e][:])

            # 9th-largest cosine threshold -> weights -> output matmul
            grid = p_small.tile([K, C], FP32, tag="grid", name=f"grid{e}")
            g8 = p_small.tile([K, 8], FP32, tag="g8", name=f"g8{e}")
            gscr = p_small.tile([K, C], FP32, tag="gscr", name=f"gscr{e}")
            g8b = p_small.tile([K, 8], FP32, tag="g8b", name=f"g8b{e}")
            mask = p_small.tile([K, C], FP32, tag="mask", name=f"mask{e}")
            m_col = p_small.tile([KC, 1], FP32, tag="m_col", name=f"m_col{e}")
            w_col = p_small.tile([KC, 1], FP32, tag="w_col", name=f"w_col{e}")
            bd = p_small.tile([KC, K], FP32, tag="bd", name=f"bd{e}")
            nc.scalar.dma_start(out=grid[:], in_=dots[e][:])
            nc.vector.max(g8[:], grid[:])
            nc.vector.match_replace(gscr[:], g8[:], grid[:], NEG_BIG)
            nc.vector.max(g8b[:], gscr[:])
            nc.vector.tensor_tensor(
                mask[:], grid[:], g8b[:, 0:1].to_broadcast([K, C]),
                op=mybir.AluOpType.is_ge)
            if dbg:
                nc.sync.dma_start(out=dbg_mask.ap()[e], in_=mask[:])
            nc.scalar.dma_start(out=m_col[:], in_=mask[:])
            nc.vector.scalar_tensor_tensor(
                w_col[:], m_col[:], 1.0 / 9.0, nrm[e][:],
                op0=mybir.AluOpType.mult, op1=mybir.AluOpType.mult)
            nc.vector.tensor_tensor(
                bd[:], bdm[:], w_col[:, 0:1].to_broadcast([KC, K]),
                op=mybir.AluOpType.mult)
            for h in range(D // 512):
                ps_o = p_ps_m.tile([K, 512], FP32, space="PSUM", tag="ps_misc")
                nc.tensor.matmul(
                    ps_o[:], bd[:], gath[e][:, 512 * h:512 * (h + 1)],
                    start=True, stop=True,
                )
                if h % 2 == 0:
                    nc.vector.tensor_copy(outsb[e][:, 512 * h:512 * (h + 1)], ps_o[:])
                else:
                    nc.scalar.copy(outsb[e][:, 512 * h:512 * (h + 1)], ps_o[:])
            nc.sync.dma_start(out=out_d.ap()[e], in_=outsb[e][:])

        def _sizes(e):
            # split the very first and very last super-tiles in half to
            # shorten pipeline fill (compute starts after 2 MB, not 4 MB)
            # and drain (last tile's compute+tail covers half the data)
            full = [NSUP] * NSUPS
            if NSUP >= 1024 and not coalesce:
                if e == 0:
                    full = [NSUP // 2, NSUP // 2] + full[1:]
                if e == EB - 1:
                    full = full[:-1] + [NSUP // 2, NSUP // 2]
            return full

        def _body(_i=None):
            if interleave == "fixed":
                # overlap e0's tail under e1's stream, but issue e0's gather
                # only after e1's stream DMAs are enqueued on the Pool queue
                _stream(0, _sizes(0))
                if tail:
                    _tail_merge(0)
                _stream(1, _sizes(1))
                if tail:
                    _tail_rest(0)
                    _tail_merge(1)
                    _tail_rest(1)
            elif interleave:
                for e in range(EB):
                    _stream(e, _sizes(e))
                    if tail:
                        _tail_merge(e)
                        _tail_rest(e)
            else:
                for e in range(EB):
                    _stream(e, _sizes(e))
                if tail:
                    for e in range(EB):
                        _tail_merge(e)
                        _tail_rest(e)

        if loop_iters:
            with tc.For_i(0, loop_iters, 1) as _it:
                _body(_it)
        else:
            _body()

    nc.compile()
    if split:
        _split_multiwaits(nc)
    return nc


def kernel(cue: np.ndarray, patches: np.ndarray) -> np.ndarray:
    global LAST_EXEC_NS
    cue = np.ascontiguousarray(cue, dtype=np.float32)
    patches = np.ascontiguousarray(patches, dtype=np.float32)
    assert cue.shape == (B, K, D) and patches.shape == (B, N, D)

    if "nc" not in _CACHE:
        _CACHE["nc"] = _build_kernel()
    nc = _CACHE["nc"]

    in_maps = [
        {
            "cue": cue[EB * i:EB * (i + 1)],
            "patches": patches[EB * i:EB * (i + 1)],
        }
        for i in range(NCORES)
    ]
    res = run_bass_kernel_spmd(nc, in_maps, core_ids=list(range(NCORES)))
    LAST_EXEC_NS = res.exec_time_ns
    out = np.concatenate([res.results[i]["out"] for i in range(NCORES)], axis=0)
    return out.astype(np.float32)

